# revision 68
# baseline (speedup 1.0000x reference)
"""Causal self-attention (12 heads, T=1024, C=768, prefix P=4) on 8 TRN2 cores.

Sharding: data-parallel over batch B=8 -> one batch element per NeuronCore.
No collectives. Weights are replicated to every core.

Per-core kernel (all fp32 accumulation, fp16 operands):
  qkv projection from chunk-major tiles (xt [128,6,1024], wq [128,6,2,128]):
    qT pair tile [128, T]  (head 2p rows 0:64, head 2p+1 rows 64:128)
    kpair tile   [128, T]  (same row split)
    v            [128, H, 128] per t-chunk (col 64 = 1.0 -> softmax denom)
  scores^T tile (r, window c): TWO concurrent K=64 row-group matmuls
    (head s uses PE rows 64s..64s+63) -> psum [128kv, <=512t] each
    e = exp(0.125 * psum); diagonal band tiles masked by a 128x128
    triangular 0/1 mask; fully-masked columns never computed.
  prefix scores: ONE matmul per (p, c) via a block layout kTc8 [128, 36]
    (head0 k in rows 0:64 cols 0:4, head1 in rows 64:128 cols 32:36)
    -> pp [36, W]; single exp -> etp8; AV-prefix matmuls use row groups
    0 / 32 so the two heads' prefix updates run concurrently.
  AV: py[0:65, t] = [y(64 dims); denom] accumulated over kv chunks.
  norm (no DRAM round trips): ScalarE copies py -> sb [65, W] SBUF,
    DVE reciprocal in-place on sb row 64, PE K=1 f32r matmul broadcasts
    the reciprocal row to 64 partitions, DVE multiply -> yT.
  out = yT.T @ w_proj + b_proj  -> [T, 768] -> DMA out.

Startup: 20 warmup matmuls on a zero tile keep the PE HAM warm while
the first DMAs (wq0, xt) land; DMA layouts are chunk-major so each
load is one large-line transfer.
"""

import numpy as np
from contextlib import ExitStack

import concourse.bass as bass
import concourse.mybir as mybir
import concourse.tile as tile
from concourse import bacc
from concourse.bass_utils import run_bass_kernel_spmd

F32 = mybir.dt.float32
F32R = mybir.dt.float32r
F16 = mybir.dt.float16
N_CORES = 8
T, C, H, D, PFX = 1024, 768, 12, 64, 4
NPAIR = H // 2          # 6 head pairs
KC = C // 128           # 6 contraction chunks
W = 512                 # T window for scores
NW = T // W             # 2 windows
TCH = T // 128          # 8 T chunks
EXP = mybir.ActivationFunctionType.Exp
COPY = mybir.ActivationFunctionType.Copy
SCALE = 1.0 / np.sqrt(D)


def _build():
    nc = bacc.Bacc("TRN2", target_bir_lowering=False, debug=False,
                   num_devices=N_CORES)
    xt_d = nc.declare_dram_parameter("xt", [128, KC, T], F16, isOutput=False)
    wq_d = [nc.declare_dram_parameter(f"wq{p}", [128, KC, 2, 128], F16,
                                      isOutput=False) for p in range(NPAIR)]
    wv_d = nc.declare_dram_parameter("wv", [128, KC, C], F16, isOutput=False)
    wp_d = nc.declare_dram_parameter("wp", [128, KC, C], F16, isOutput=False)
    bqk_d = nc.declare_dram_parameter("b_qk", [128, 12], F32, isOutput=False)
    bv_d = nc.declare_dram_parameter("bv_bc", [128, C], F32, isOutput=False)
    bp_d = nc.declare_dram_parameter("bp_bc", [128, C], F32, isOutput=False)
    ktc_d = nc.declare_dram_parameter("kTc8", [128, NPAIR, 36], F16,
                                      isOutput=False)
    vcp_d = nc.declare_dram_parameter("vcP", [64, H, 128], F16, isOutput=False)
    tri_d = nc.declare_dram_parameter("tri", [128, 128], F16, isOutput=False)
    ones_d = nc.declare_dram_parameter("ones2", [2, 128], F16, isOutput=False)
    out_d = nc.declare_dram_parameter("out", [T, C], F16, isOutput=True)

    with tile.TileContext(nc) as tc, ExitStack() as ctx:
        pers = ctx.enter_context(tc.tile_pool(name="pers", bufs=1))
        wqp = ctx.enter_context(tc.tile_pool(name="wqp", bufs=6))
        qkp = ctx.enter_context(tc.tile_pool(name="qkp", bufs=2))
        khp = ctx.enter_context(tc.tile_pool(name="khp", bufs=4))
        ep = ctx.enter_context(tc.tile_pool(name="ep", bufs=13))
        epp = ctx.enter_context(tc.tile_pool(name="epp", bufs=6))
        sbp = ctx.enter_context(tc.tile_pool(name="sbp", bufs=6))
        rwp = ctx.enter_context(tc.tile_pool(name="rwp", bufs=2))
        drp = ctx.enter_context(tc.tile_pool(name="drp", bufs=2))
        dram = ctx.enter_context(tc.tile_pool(name="dram", bufs=3,
                                              space="DRAM"))
        op = ctx.enter_context(tc.tile_pool(name="op", bufs=2))
        ps = ctx.enter_context(tc.tile_pool(name="ps", bufs=4, space="PSUM"))
        psp = ctx.enter_context(tc.tile_pool(name="psp", bufs=1, space="PSUM"))
        pyp = ctx.enter_context(tc.tile_pool(name="pyp", bufs=2, space="PSUM"))
        pbp = ctx.enter_context(tc.tile_pool(name="pbp", bufs=1, space="PSUM"))

        # ---- PE warmup: HAM needs ~3.4us of activity to unthrottle ------
        wtile = pers.tile([128, W], F16, tag="wtile")
        nc.vector.memset(wtile[:], 0.0)
        pwarm = ps.tile([128, 512], F32, tag="ps", name="pwarm")
        for i in range(20):
            nc.tensor.matmul(pwarm[:], wtile[:, 0:128], wtile[:],
                             start=True, stop=True)

        # ---- persistent loads, priority order ---------------------------
        wq = [None] * NPAIR
        wq[0] = wqp.tile([128, KC, 2, 128], F16, tag="wq", name="wq0")
        xt = pers.tile([128, KC, T], F16, tag="xt")
        for k in range(KC):  # interleaved per-chunk so the first qkproj
            # matmul only waits for (wq0[0], xt[0])
            nc.sync.dma_start(wq[0][:, k, :, :], wq_d[0][:, k, :, :])
            nc.sync.dma_start(xt[:, k, :], xt_d[:, k, :])
        bqk = pers.tile([128, 12], F32, tag="bqk")
        nc.sync.dma_start(bqk[:], bqk_d[:])
        wv = pers.tile([128, KC, C], F16, tag="wv")
        nc.sync.dma_start(wv[:], wv_d[:])
        bv = pers.tile([128, C], F32, tag="bv")
        nc.sync.dma_start(bv[:], bv_d[:])
        tri = pers.tile([128, 128], F16, tag="tri")
        nc.sync.dma_start(tri[:], tri_d[:])
        ktc = pers.tile([128, NPAIR, 36], F16, tag="ktc")
        nc.sync.dma_start(ktc[:], ktc_d[:])
        vcp = pers.tile([64, H, 128], F16, tag="vcp")
        nc.sync.dma_start(vcp[:], vcp_d[:])
        for p in range(1, NPAIR):
            wq[p] = wqp.tile([128, KC, 2, 128], F16, tag="wq", name=f"wq{p}")
            nc.sync.dma_start(wq[p][:], wq_d[p][:])

        wp = pers.tile([128, KC, C], F16, tag="wp")
        nc.sync.dma_start(wp[:], wp_d[:])
        bp = pers.tile([128, C], F32, tag="bp")
        nc.sync.dma_start(bp[:], bp_d[:])

        ones1 = pers.tile([66, 128], F16, tag="ones1")
        nc.sync.dma_start(ones1[64:66, :], ones_d[:])

        yT = [pers.tile([128, T], F16, tag=f"yT{p}", name=f"yT{p}")
              for p in range(NPAIR)]

        # ---- phases ----
        qk_tiles = {}
        ets = {}
        etps = {}
        pys = {}

        def qkproj(p, halves=(0, 1)):
            if p in qk_tiles:
                qT, kh = qk_tiles[p]
            else:
                qT = qkp.tile([128, T], F16, tag="qT", name=f"qT{p}")
                kh = [khp.tile([128, T], F16, tag="kh", name=f"kh{p}_{s}")
                      for s in range(2)]
                qk_tiles[p] = (qT, kh)
                for s in range(2):
                    # head s's k-features live at rows 64s..64s+63 (aligned
                    # with its q rows in the pair tile); other 64 rows zero.
                    nc.gpsimd.memset(kh[s][64 - 64 * s:128 - 64 * s, :], 0.0)
            for half in halves:
                for w in range(NW):
                    pq = ps.tile([128, 512], F32, tag="ps",
                                 name=f"pq{p}_{half}_{w}")
                    for k in range(KC):
                        nc.tensor.matmul(pq[:], wq[p][:, k, half, :],
                                         xt[:, k, W * w:W * w + W],
                                         start=(k == 0), stop=(k == KC - 1))
                    if half == 0:
                        nc.scalar.activation(qT[:, W * w:W * w + W], pq[:],
                                             COPY, bias=bqk[:, p:p + 1])
                    else:
                        for s in range(2):
                            nc.vector.tensor_scalar_add(
                                kh[s][64 * s:64 * s + 64, W * w:W * w + W],
                                pq[64 * s:64 * s + 64, :],
                                bqk[64 * s:64 * s + 64, 6 + p:7 + p])

        def vproj():
            vt = []
            for mt in range(TCH):
                v_ = pers.tile([128, H, 128], F16, tag=f"v{mt}")
                nc.gpsimd.memset(v_[:, :, 64:65], 1.0)
                nc.gpsimd.memset(v_[:, :, 65:128], 0.0)
                for n0, nsz in ((0, 512), (512, 256)):
                    pv = ps.tile([128, 512], F32, tag="ps", name=f"pv{mt}_{n0}")
                    for k in range(KC):
                        nc.tensor.matmul(pv[:, :nsz],
                                         xt[:, k, 128 * mt:128 * mt + 128],
                                         wv[:, k, n0:n0 + nsz],
                                         start=(k == 0), stop=(k == KC - 1))
                    h0, hn = n0 // 64, nsz // 64
                    nc.vector.tensor_add(
                        v_[:, h0:h0 + hn, 0:64],
                        pv[:, :nsz].rearrange("a (h d) -> a h d", d=64),
                        bv[:, n0:n0 + nsz].rearrange("a (h d) -> a h d", d=64))
                vt.append(v_)
            return vt

        def scores(p):
            """Both heads of pair p: concurrent K=64 row-group matmuls."""
            qT, kh = qk_tiles[p]
            for c in range(NW):
                # prefix first: both heads in one matmul via the kTc8 block
                # layout; its exp lands early in the scalar queue so the
                # AV-prefix matmuls never stall on it.
                pp = psp.tile([36, 512], F32, tag="psp", name=f"pp{p}_{c}")
                nc.tensor.matmul(pp[:], ktc[:, p, :],
                                 qT[:, W * c:W * (c + 1)], start=True,
                                 stop=True)
                ep_ = epp.tile([36, W], F16, tag="etp", name=f"etp{p}_{c}")
                nc.scalar.activation(ep_[:], pp[:], EXP, scale=float(SCALE))
                etps[(p, c)] = ep_
                for r in range(4 * c + 4):
                    # both heads' e for (c, r) share one [128, 2, W] tile
                    e2 = ep.tile([128, 2, W], F16, tag="et",
                                 name=f"et{p}_{c}_{r}")
                    ets[(p, c, r)] = e2
                    j0 = 128 * r - W * c if r >= 4 * c else 0
                    for s in range(2):
                        pss = ps.tile([128, 512], F32, tag="ps",
                                      name=f"pss{p}_{s}_{c}_{r}")
                        nc.tensor.matmul(
                            pss[:, j0:W],
                            kh[s][:, 128 * r:128 * r + 128],
                            qT[:, W * c + j0:W * (c + 1)],
                            start=True, stop=True)
                        nc.scalar.activation(e2[:, s, j0:W], pss[:, j0:W],
                                             EXP, scale=float(SCALE))
                    if r >= 4 * c:  # one masked multiply covers both heads
                        nc.vector.tensor_mul(
                            e2[:, :, j0:j0 + 128], e2[:, :, j0:j0 + 128],
                            tri[:].unsqueeze(1).broadcast_to((128, 2, 128)))

        def av(p, c, vt):
            """y^T accumulation for both heads: py[0:65, t] = [y; denom]."""
            py = {}
            for s in range(2):
                py[s] = pyp.tile([128, W], F32, tag="py", name=f"py{p}_{s}_{c}")
                pys[(p, s, c)] = py[s]
            # prefix first: its exp is the earliest scalar op of the window,
            # so these never wait; they open the accumulation group.
            for s in range(2):
                nc.tensor.matmul(py[s][:, :],
                                 vcp[32 * s:32 * s + 4, 2 * p + s, :],
                                 etps[(p, c)][32 * s:32 * s + 4, :],
                                 start=True, stop=False)
            last = 4 * c + 3
            for r in range(4 * c + 4):
                tstart = max(0, 128 * r - W * c)
                for s in range(2):
                    nc.tensor.matmul(py[s][:, tstart:W],
                                     vt[r][:, 2 * p + s, :],
                                     ets[(p, c, r)][:, s, tstart:W],
                                     start=False, stop=(r == last))

        def norm_pre(p, c):
            """Launch the denominator reciprocal chain for window c.

            The two denom rows [1, W] are DMA'd to DRAM, reloaded as
            [64, 16] so the reciprocal runs lane-parallel, converted to
            f16 and DMA'd back as a [2, W] row pair at partitions 64:66."""
            sbs = []
            dstage = dram.tile([2, W], F32, tag="dst", name=f"dst{p}_{c}")
            for s in range(2):
                py = pys[(p, s, c)]
                sb = sbp.tile([65, W], F32, tag="sb", name=f"sb{p}_{s}_{c}")
                nc.vector.tensor_copy(sb[:], py[0:65, :])
                nc.sync.dma_start(dstage[s:s + 1, :], sb[64:65, :])
                sbs.append(sb)
            dT = drp.tile([64, 16], F32, tag="dT", name=f"dT{p}_{c}")
            nc.sync.dma_start(
                dT[:], dstage[:].rearrange("r (q f) -> (r q) f", f=16))
            rT = drp.tile([64, 16], F32, tag="rT", name=f"rT{p}_{c}")
            nc.vector.reciprocal(rT[:], dT[:])
            rT16 = drp.tile([64, 16], F16, tag="rT16", name=f"rF{p}_{c}")
            with nc.allow_low_precision(reason="softmax denom recip f16"):
                nc.vector.tensor_copy(rT16[:], rT[:])
            d2 = dram.tile([2, W], F16, tag="d2", name=f"d2{p}_{c}")
            nc.sync.dma_start(
                d2[:].rearrange("r (q f) -> (r q) f", f=16), rT16[:])
            rrow = rwp.tile([66, W], F16, tag="rrow", name=f"rw{p}_{c}")
            nc.sync.dma_start(rrow[64:66, :], d2[:])
            return sbs, rrow

        def norm_mul(p, c, sbs, rrow):
            """Broadcast 1/D of BOTH heads in one K=2 f16 matmul (rows
            0:64 head 0, rows 64:128 head 1) and multiply into yT."""
            pb = pbp.tile([128, W], F32, tag="pb", name=f"pb{p}_{c}")
            nc.tensor.matmul(pb[:], ones1[64:66, :], rrow[64:66, :],
                             start=True, stop=True)
            for s in range(2):
                nc.vector.tensor_mul(yT[p][64 * s:64 * s + 64,
                                           W * c:W * c + W],
                                     sbs[s][0:64, :],
                                     pb[64 * s:64 * s + 64, :])

        def norm(p, c):
            norm_mul(p, c, *norm_pre(p, c))

        def outproj(mts, kps=range(NPAIR)):
            kps = list(kps)
            for mt in mts:
                osb = op.tile([128, C], F16, tag="osb", name=f"osb{mt}")
                for n0, nsz in ((0, 512), (512, 256)):
                    po = ps.tile([128, 512], F32, tag="ps", name=f"po{mt}_{n0}")
                    for kp in kps:
                        nc.tensor.matmul(po[:, :nsz],
                                         yT[kp][:, 128 * mt:128 * mt + 128],
                                         wp[:, kp, n0:n0 + nsz],
                                         start=(kp == kps[0]),
                                         stop=(kp == kps[-1]))
                    nc.vector.tensor_add(osb[:, n0:n0 + nsz], po[:, :nsz],
                                         bp[:, n0:n0 + nsz])
                nc.sync.dma_start(out_d[128 * mt:128 * mt + 128, :], osb[:])

        def outproj_hold(mt, pool, tag):
            """Accumulate pairs 0..4 for chunk mt, hold the psum open."""
            pos = []
            for n0, nsz in ((0, 512), (512, 256)):
                po = pool.tile([128, 512], F32, tag=tag, name=f"ph{mt}_{n0}")
                for kp in range(NPAIR - 1):
                    nc.tensor.matmul(po[:, :nsz],
                                     yT[kp][:, 128 * mt:128 * mt + 128],
                                     wp[:, kp, n0:n0 + nsz],
                                     start=(kp == 0), stop=False)
                pos.append((n0, nsz, po))
            return pos

        def outproj_finish(mt, pos):
            osb = op.tile([128, C], F16, tag="osb", name=f"osb{mt}")
            for n0, nsz, po in pos:
                nc.tensor.matmul(po[:, :nsz],
                                 yT[NPAIR - 1][:, 128 * mt:128 * mt + 128],
                                 wp[:, NPAIR - 1, n0:n0 + nsz],
                                 start=False, stop=True)
                nc.vector.tensor_add(osb[:, n0:n0 + nsz], po[:, :nsz],
                                     bp[:, n0:n0 + nsz])
            nc.sync.dma_start(out_d[128 * mt:128 * mt + 128, :], osb[:])

        # ---- emission schedule ----
        qkproj(0)
        vt = vproj()
        scores(0)
        for p in range(NPAIR - 1):
            # scalar-independent PE work interleaved before each av phase
            # lets the exp queue catch up before the AV matmuls consume it
            qkproj(p + 1, halves=(0,))
            av(p, 0, vt)
            norm(p, 0)
            qkproj(p + 1, halves=(1,))
            av(p, 1, vt)
            norm(p, 1)
            scores(p + 1)
        # last pair: both norm chains launch concurrently; held partial
        # accumulations (pairs 0..4) keep the PE busy while they drain,
        # so only the tiny pair-5 matmuls wait on the final norm.
        pl = NPAIR - 1
        av(pl, 0, vt)
        n0state = norm_pre(pl, 0)
        av(pl, 1, vt)
        n1state = norm_pre(pl, 1)
        norm_mul(pl, 0, *n0state)
        outproj(range(0, 4))
        held = [(4, outproj_hold(4, pyp, "py")),
                (5, outproj_hold(5, ps, "ps"))]
        norm_mul(pl, 1, *n1state)
        for mt, pos in held:
            outproj_finish(mt, pos)
        outproj(range(6, TCH))

    nc.finalize()
    return nc


def _prep_inputs(x, kv_cvec, w_attn, b_attn, w_proj, b_proj):
    x = np.asarray(x, np.float32)
    kv_cvec = np.asarray(kv_cvec, np.float32)
    w_attn = np.asarray(w_attn, np.float32)
    b_attn = np.asarray(b_attn, np.float32)
    w_proj = np.asarray(w_proj, np.float32)
    b_proj = np.asarray(b_proj, np.float32)

    def chunk_major(w):  # [C, N] -> [128, KC, N]
        return np.ascontiguousarray(
            w.reshape(KC, 128, w.shape[1]).transpose(1, 0, 2))

    shared = {
        "wv": chunk_major(w_attn[:, 2 * C:]).astype(np.float16),
        "wp": chunk_major(w_proj).astype(np.float16),
        "b_qk": np.ascontiguousarray(b_attn[:2 * C].reshape(12, 128).T),
        "bv_bc": np.ascontiguousarray(
            np.broadcast_to(b_attn[2 * C:], (128, C))),
        "bp_bc": np.ascontiguousarray(np.broadcast_to(b_proj, (128, C))),
        "tri": (np.arange(128)[:, None] <= np.arange(128)[None, :]
                ).astype(np.float16),
        "ones2": np.kron(np.eye(2), np.ones((1, 64))).astype(np.float16),
    }
    for p in range(NPAIR):
        wqp = np.stack([w_attn[:, 128 * p:128 * p + 128],
                        w_attn[:, C + 128 * p:C + 128 * p + 128]], axis=1)
        shared[f"wq{p}"] = chunk_major(
            wqp.reshape(C, 256)).reshape(128, KC, 2, 128).astype(np.float16)

    in_maps = []
    for b in range(N_CORES):
        kc = kv_cvec[b][:, :C].reshape(PFX, H, D)      # [j, h, d]
        vc = kv_cvec[b][:, C:].reshape(PFX, H, D)
        ktc8 = np.zeros((128, NPAIR, 36), np.float32)
        for s in range(2):
            # [d, p, j] block for head 2p+s
            ktc8[64 * s:64 * s + 64, :, 32 * s:32 * s + 4] = \
                kc[:, s::2, :].transpose(2, 1, 0)
        vcp = np.zeros((64, H, 128), np.float32)
        for s0 in (0, 32):
            vcp[s0:s0 + 4, :, :64] = vc
            vcp[s0:s0 + 4, :, 64] = 1.0
        m = dict(shared)
        m["xt"] = np.ascontiguousarray(
            x[b].T.reshape(KC, 128, T).transpose(1, 0, 2)).astype(np.float16)
        m["kTc8"] = ktc8.astype(np.float16)
        m["vcP"] = vcp.astype(np.float16)
        in_maps.append(m)
    return in_maps


_NC_CACHE = {}


def run_hw(trace=False, **inputs):
    """Build+compile+run on 8 NeuronCores; returns (out [8,1024,768], results)."""
    if "nc" not in _NC_CACHE:
        _NC_CACHE["nc"] = _build()
    nc = _NC_CACHE["nc"]
    in_maps = _prep_inputs(**inputs)
    res = run_bass_kernel_spmd(nc, in_maps, list(range(N_CORES)), trace=trace)
    out = np.stack([res.results[b]["out"].astype(np.float32)
                    for b in range(N_CORES)])
    return out, res


def kernel(**inputs):
    out, _ = run_hw(trace=False, **inputs)
    return out


# revision 71
# speedup vs baseline: 1.0152x; 1.0152x over previous
"""Causal self-attention (12 heads, T=1024, C=768, prefix P=4) on 8 TRN2 cores.

Sharding: data-parallel over batch B=8 -> one batch element per NeuronCore.
No collectives. Weights are replicated to every core.

Per-core kernel (all fp32 accumulation, fp16 operands):
  qkv projection from chunk-major tiles (xt [128,6,1024], wq [128,6,2,128]):
    qT pair tile [128, T]  (head 2p rows 0:64, head 2p+1 rows 64:128)
    kpair tile   [128, T]  (same row split)
    v            [128, H, 128] per t-chunk (col 64 = 1.0 -> softmax denom)
  scores^T tile (r, window c): TWO concurrent K=64 row-group matmuls
    (head s uses PE rows 64s..64s+63) -> psum [128kv, <=512t] each
    e = exp(0.125 * psum); diagonal band tiles masked by a 128x128
    triangular 0/1 mask; fully-masked columns never computed.
  prefix scores: ONE matmul per (p, c) via a block layout kTc8 [128, 36]
    (head0 k in rows 0:64 cols 0:4, head1 in rows 64:128 cols 32:36)
    -> pp [36, W]; single exp -> etp8; AV-prefix matmuls use row groups
    0 / 32 so the two heads' prefix updates run concurrently.
  AV: py[0:65, t] = [y(64 dims); denom] accumulated over kv chunks.
  norm (no DRAM round trips): ScalarE copies py -> sb [65, W] SBUF,
    DVE reciprocal in-place on sb row 64, PE K=1 f32r matmul broadcasts
    the reciprocal row to 64 partitions, DVE multiply -> yT.
  out = yT.T @ w_proj + b_proj  -> [T, 768] -> DMA out.

Startup: 20 warmup matmuls on a zero tile keep the PE HAM warm while
the first DMAs (wq0, xt) land; DMA layouts are chunk-major so each
load is one large-line transfer.
"""

import numpy as np
from contextlib import ExitStack

import concourse.bass as bass
import concourse.mybir as mybir
import concourse.tile as tile
from concourse import bacc
from concourse.bass_utils import run_bass_kernel_spmd

F32 = mybir.dt.float32
F32R = mybir.dt.float32r
F16 = mybir.dt.float16
N_CORES = 8
T, C, H, D, PFX = 1024, 768, 12, 64, 4
NPAIR = H // 2          # 6 head pairs
KC = C // 128           # 6 contraction chunks
W = 512                 # T window for scores
NW = T // W             # 2 windows
TCH = T // 128          # 8 T chunks
EXP = mybir.ActivationFunctionType.Exp
COPY = mybir.ActivationFunctionType.Copy
SCALE = 1.0 / np.sqrt(D)


def _build():
    nc = bacc.Bacc("TRN2", target_bir_lowering=False, debug=False,
                   num_devices=N_CORES)
    xt_d = nc.declare_dram_parameter("xt", [128, KC, T], F16, isOutput=False)
    wq_d = [nc.declare_dram_parameter(f"wq{p}", [128, KC, 2, 128], F16,
                                      isOutput=False) for p in range(NPAIR)]
    wv_d = nc.declare_dram_parameter("wv", [128, KC, C], F16, isOutput=False)
    wp_d = nc.declare_dram_parameter("wp", [128, KC, C], F16, isOutput=False)
    bqk_d = nc.declare_dram_parameter("b_qk", [128, 12], F32, isOutput=False)
    bv_d = nc.declare_dram_parameter("bv_bc", [128, C], F32, isOutput=False)
    bp_d = nc.declare_dram_parameter("bp_bc", [128, C], F32, isOutput=False)
    ktc_d = nc.declare_dram_parameter("kTc8", [128, NPAIR, 36], F16,
                                      isOutput=False)
    vcp_d = nc.declare_dram_parameter("vcP", [64, H, 128], F16, isOutput=False)
    tri_d = nc.declare_dram_parameter("tri", [128, 128], F16, isOutput=False)
    ones_d = nc.declare_dram_parameter("ones2", [2, 128], F16, isOutput=False)
    out_d = nc.declare_dram_parameter("out", [T, C], F16, isOutput=True)

    with tile.TileContext(nc) as tc, ExitStack() as ctx:
        pers = ctx.enter_context(tc.tile_pool(name="pers", bufs=1))
        wqp = ctx.enter_context(tc.tile_pool(name="wqp", bufs=6))
        qkp = ctx.enter_context(tc.tile_pool(name="qkp", bufs=2))
        khp = ctx.enter_context(tc.tile_pool(name="khp", bufs=4))
        ep = ctx.enter_context(tc.tile_pool(name="ep", bufs=13))
        epp = ctx.enter_context(tc.tile_pool(name="epp", bufs=6))
        sbp = ctx.enter_context(tc.tile_pool(name="sbp", bufs=6))
        rwp = ctx.enter_context(tc.tile_pool(name="rwp", bufs=2))
        drp = ctx.enter_context(tc.tile_pool(name="drp", bufs=2))
        dram = ctx.enter_context(tc.tile_pool(name="dram", bufs=3,
                                              space="DRAM"))
        op = ctx.enter_context(tc.tile_pool(name="op", bufs=2))
        ps = ctx.enter_context(tc.tile_pool(name="ps", bufs=4, space="PSUM"))
        psp = ctx.enter_context(tc.tile_pool(name="psp", bufs=1, space="PSUM"))
        pyp = ctx.enter_context(tc.tile_pool(name="pyp", bufs=2, space="PSUM"))
        pbp = ctx.enter_context(tc.tile_pool(name="pbp", bufs=1, space="PSUM"))

        # ---- PE warmup: HAM needs ~3.4us of activity to unthrottle ------
        wtile = pers.tile([128, W], F16, tag="wtile")
        nc.vector.memset(wtile[:], 0.0)
        pwarm = ps.tile([128, 512], F32, tag="ps", name="pwarm")
        for i in range(20):
            nc.tensor.matmul(pwarm[:], wtile[:, 0:128], wtile[:],
                             start=True, stop=True)

        # ---- persistent loads, priority order ---------------------------
        wq = [None] * NPAIR
        wq[0] = wqp.tile([128, KC, 2, 128], F16, tag="wq", name="wq0")
        xt = pers.tile([128, KC, T], F16, tag="xt")
        for k in range(KC):  # interleaved per-chunk so the first qkproj
            # matmul only waits for (wq0[0], xt[0])
            nc.sync.dma_start(wq[0][:, k, :, :], wq_d[0][:, k, :, :])
            nc.sync.dma_start(xt[:, k, :], xt_d[:, k, :])
        bqk = pers.tile([128, 12], F32, tag="bqk")
        nc.sync.dma_start(bqk[:], bqk_d[:])
        wv = pers.tile([128, KC, C], F16, tag="wv")
        nc.sync.dma_start(wv[:], wv_d[:])
        bv = pers.tile([128, C], F32, tag="bv")
        nc.sync.dma_start(bv[:], bv_d[:])
        tri = pers.tile([128, 128], F16, tag="tri")
        nc.sync.dma_start(tri[:], tri_d[:])
        ktc = pers.tile([128, NPAIR, 36], F16, tag="ktc")
        nc.sync.dma_start(ktc[:], ktc_d[:])
        vcp = pers.tile([64, H, 128], F16, tag="vcp")
        nc.sync.dma_start(vcp[:], vcp_d[:])
        for p in range(1, NPAIR):
            wq[p] = wqp.tile([128, KC, 2, 128], F16, tag="wq", name=f"wq{p}")
            nc.sync.dma_start(wq[p][:], wq_d[p][:])

        wp = pers.tile([128, KC, C], F16, tag="wp")
        nc.sync.dma_start(wp[:], wp_d[:])
        bp = pers.tile([128, C], F32, tag="bp")
        nc.sync.dma_start(bp[:], bp_d[:])

        ones1 = pers.tile([66, 128], F16, tag="ones1")
        nc.sync.dma_start(ones1[64:66, :], ones_d[:])

        yT = [pers.tile([128, T], F16, tag=f"yT{p}", name=f"yT{p}")
              for p in range(NPAIR)]

        # ---- phases ----
        qk_tiles = {}
        ets = {}
        etps = {}
        pys = {}

        def qkproj(p, groups=((0, 0), (0, 1), (1, 0), (1, 1))):
            if p in qk_tiles:
                qT, kh = qk_tiles[p]
            else:
                qT = qkp.tile([128, T], F16, tag="qT", name=f"qT{p}")
                kh = [khp.tile([128, T], F16, tag="kh", name=f"kh{p}_{s}")
                      for s in range(2)]
                qk_tiles[p] = (qT, kh)
                for s in range(2):
                    # head s's k-features live at rows 64s..64s+63 (aligned
                    # with its q rows in the pair tile); other 64 rows zero.
                    nc.gpsimd.memset(kh[s][64 - 64 * s:128 - 64 * s, :], 0.0)
            for half, w in groups:
                pq = ps.tile([128, 512], F32, tag="ps",
                             name=f"pq{p}_{half}_{w}")
                for k in range(KC):
                    nc.tensor.matmul(pq[:], wq[p][:, k, half, :],
                                     xt[:, k, W * w:W * w + W],
                                     start=(k == 0), stop=(k == KC - 1))
                if half == 0:
                    nc.scalar.activation(qT[:, W * w:W * w + W], pq[:],
                                         mybir.ActivationFunctionType.Identity,
                                         bias=bqk[:, p:p + 1])
                else:
                    for s in range(2):
                        nc.vector.tensor_scalar_add(
                            kh[s][64 * s:64 * s + 64, W * w:W * w + W],
                            pq[64 * s:64 * s + 64, :],
                            bqk[64 * s:64 * s + 64, 6 + p:7 + p])

        def vproj():
            vt = []
            for mt in range(TCH):
                v_ = pers.tile([128, H, 128], F16, tag=f"v{mt}")
                nc.gpsimd.memset(v_[:, :, 64:65], 1.0)
                nc.gpsimd.memset(v_[:, :, 65:128], 0.0)
                for n0, nsz in ((0, 512), (512, 256)):
                    pv = ps.tile([128, 512], F32, tag="ps", name=f"pv{mt}_{n0}")
                    for k in range(KC):
                        nc.tensor.matmul(pv[:, :nsz],
                                         xt[:, k, 128 * mt:128 * mt + 128],
                                         wv[:, k, n0:n0 + nsz],
                                         start=(k == 0), stop=(k == KC - 1))
                    h0, hn = n0 // 64, nsz // 64
                    nc.vector.tensor_add(
                        v_[:, h0:h0 + hn, 0:64],
                        pv[:, :nsz].rearrange("a (h d) -> a h d", d=64),
                        bv[:, n0:n0 + nsz].rearrange("a (h d) -> a h d", d=64))
                vt.append(v_)
            return vt

        def scores(p):
            """Both heads of pair p: concurrent K=64 row-group matmuls."""
            qT, kh = qk_tiles[p]
            for c in range(NW):
                # prefix first: both heads in one matmul via the kTc8 block
                # layout; its exp lands early in the scalar queue so the
                # AV-prefix matmuls never stall on it.
                pp = psp.tile([36, 512], F32, tag="psp", name=f"pp{p}_{c}")
                nc.tensor.matmul(pp[:], ktc[:, p, :],
                                 qT[:, W * c:W * (c + 1)], start=True,
                                 stop=True)
                ep_ = epp.tile([36, W], F16, tag="etp", name=f"etp{p}_{c}")
                nc.scalar.activation(ep_[:], pp[:], EXP, scale=float(SCALE))
                etps[(p, c)] = ep_
                for r in range(4 * c + 4):
                    # both heads' e for (c, r) share one [128, 2, W] tile
                    e2 = ep.tile([128, 2, W], F16, tag="et",
                                 name=f"et{p}_{c}_{r}")
                    ets[(p, c, r)] = e2
                    j0 = 128 * r - W * c if r >= 4 * c else 0
                    for s in range(2):
                        pss = ps.tile([128, 512], F32, tag="ps",
                                      name=f"pss{p}_{s}_{c}_{r}")
                        nc.tensor.matmul(
                            pss[:, j0:W],
                            kh[s][:, 128 * r:128 * r + 128],
                            qT[:, W * c + j0:W * (c + 1)],
                            start=True, stop=True)
                        nc.scalar.activation(e2[:, s, j0:W], pss[:, j0:W],
                                             EXP, scale=float(SCALE))
                    if r >= 4 * c:  # one masked multiply covers both heads
                        nc.vector.tensor_mul(
                            e2[:, :, j0:j0 + 128], e2[:, :, j0:j0 + 128],
                            tri[:].unsqueeze(1).broadcast_to((128, 2, 128)))

        def av(p, c, vt):
            """y^T accumulation for both heads: py[0:65, t] = [y; denom]."""
            py = {}
            for s in range(2):
                py[s] = pyp.tile([128, W], F32, tag="py", name=f"py{p}_{s}_{c}")
                pys[(p, s, c)] = py[s]
            # prefix first: its exp is the earliest scalar op of the window,
            # so these never wait; they open the accumulation group.
            for s in range(2):
                nc.tensor.matmul(py[s][:, :],
                                 vcp[32 * s:32 * s + 4, 2 * p + s, :],
                                 etps[(p, c)][32 * s:32 * s + 4, :],
                                 start=True, stop=False)
            last = 4 * c + 3
            for r in range(4 * c + 4):
                tstart = max(0, 128 * r - W * c)
                for s in range(2):
                    nc.tensor.matmul(py[s][:, tstart:W],
                                     vt[r][:, 2 * p + s, :],
                                     ets[(p, c, r)][:, s, tstart:W],
                                     start=False, stop=(r == last))

        def norm_pre(p, c):
            """Launch the denominator reciprocal chain for window c.

            The two denom rows [1, W] are DMA'd to DRAM, reloaded as
            [64, 16] so the reciprocal runs lane-parallel, converted to
            f16 and DMA'd back as a [2, W] row pair at partitions 64:66."""
            sbs = []
            dstage = dram.tile([2, W], F32, tag="dst", name=f"dst{p}_{c}")
            for s in range(2):
                py = pys[(p, s, c)]
                sb = sbp.tile([65, W], F32, tag="sb", name=f"sb{p}_{s}_{c}")
                nc.vector.tensor_copy(sb[:], py[0:65, :])
                nc.sync.dma_start(dstage[s:s + 1, :], sb[64:65, :])
                sbs.append(sb)
            dT = drp.tile([64, 16], F32, tag="dT", name=f"dT{p}_{c}")
            nc.sync.dma_start(
                dT[:], dstage[:].rearrange("r (q f) -> (r q) f", f=16))
            rT = drp.tile([64, 16], F32, tag="rT", name=f"rT{p}_{c}")
            nc.vector.reciprocal(rT[:], dT[:])
            rT16 = drp.tile([64, 16], F16, tag="rT16", name=f"rF{p}_{c}")
            with nc.allow_low_precision(reason="softmax denom recip f16"):
                nc.vector.tensor_copy(rT16[:], rT[:])
            d2 = dram.tile([2, W], F16, tag="d2", name=f"d2{p}_{c}")
            nc.sync.dma_start(
                d2[:].rearrange("r (q f) -> (r q) f", f=16), rT16[:])
            rrow = rwp.tile([66, W], F16, tag="rrow", name=f"rw{p}_{c}")
            nc.sync.dma_start(rrow[64:66, :], d2[:])
            return sbs, rrow

        def norm_mul(p, c, sbs, rrow):
            """Broadcast 1/D of BOTH heads in one K=2 f16 matmul (rows
            0:64 head 0, rows 64:128 head 1) and multiply into yT."""
            pb = pbp.tile([128, W], F32, tag="pb", name=f"pb{p}_{c}")
            nc.tensor.matmul(pb[:], ones1[64:66, :], rrow[64:66, :],
                             start=True, stop=True)
            for s in range(2):
                nc.vector.tensor_mul(yT[p][64 * s:64 * s + 64,
                                           W * c:W * c + W],
                                     sbs[s][0:64, :],
                                     pb[64 * s:64 * s + 64, :])

        def norm(p, c):
            norm_mul(p, c, *norm_pre(p, c))

        def outproj(mts, kps=range(NPAIR)):
            kps = list(kps)
            for mt in mts:
                osb = op.tile([128, C], F16, tag="osb", name=f"osb{mt}")
                for n0, nsz in ((0, 512), (512, 256)):
                    po = ps.tile([128, 512], F32, tag="ps", name=f"po{mt}_{n0}")
                    for kp in kps:
                        nc.tensor.matmul(po[:, :nsz],
                                         yT[kp][:, 128 * mt:128 * mt + 128],
                                         wp[:, kp, n0:n0 + nsz],
                                         start=(kp == kps[0]),
                                         stop=(kp == kps[-1]))
                    nc.vector.tensor_add(osb[:, n0:n0 + nsz], po[:, :nsz],
                                         bp[:, n0:n0 + nsz])
                nc.sync.dma_start(out_d[128 * mt:128 * mt + 128, :], osb[:])

        def outproj_hold(mt, pool, tag):
            """Accumulate pairs 0..4 for chunk mt, hold the psum open."""
            pos = []
            for n0, nsz in ((0, 512), (512, 256)):
                po = pool.tile([128, 512], F32, tag=tag, name=f"ph{mt}_{n0}")
                for kp in range(NPAIR - 1):
                    nc.tensor.matmul(po[:, :nsz],
                                     yT[kp][:, 128 * mt:128 * mt + 128],
                                     wp[:, kp, n0:n0 + nsz],
                                     start=(kp == 0), stop=False)
                pos.append((n0, nsz, po))
            return pos

        def outproj_finish(mt, pos):
            osb = op.tile([128, C], F16, tag="osb", name=f"osb{mt}")
            for n0, nsz, po in pos:
                nc.tensor.matmul(po[:, :nsz],
                                 yT[NPAIR - 1][:, 128 * mt:128 * mt + 128],
                                 wp[:, NPAIR - 1, n0:n0 + nsz],
                                 start=False, stop=True)
                nc.vector.tensor_add(osb[:, n0:n0 + nsz], po[:, :nsz],
                                     bp[:, n0:n0 + nsz])
            nc.sync.dma_start(out_d[128 * mt:128 * mt + 128, :], osb[:])

        # ---- emission schedule ----
        qkproj(0)
        vt = vproj()
        scores(0)
        for p in range(NPAIR - 1):
            # scalar-independent PE work interleaved before each av phase
            # lets the exp queue catch up before the AV matmuls consume it:
            # av(p,0) needs ~1.3us of cover, av(p,1) ~3.8us
            qkproj(p + 1, groups=((0, 0),))
            av(p, 0, vt)
            norm(p, 0)
            qkproj(p + 1, groups=((0, 1), (1, 0), (1, 1)))
            av(p, 1, vt)
            norm(p, 1)
            scores(p + 1)
        # last pair: both norm chains launch concurrently; held partial
        # accumulations (pairs 0..4) keep the PE busy while they drain,
        # so only the tiny pair-5 matmuls wait on the final norm.
        pl = NPAIR - 1
        av(pl, 0, vt)
        n0state = norm_pre(pl, 0)
        av(pl, 1, vt)
        n1state = norm_pre(pl, 1)
        norm_mul(pl, 0, *n0state)
        outproj(range(0, 4))
        held = [(4, outproj_hold(4, pyp, "py")),
                (5, outproj_hold(5, ps, "ps"))]
        norm_mul(pl, 1, *n1state)
        for mt, pos in held:
            outproj_finish(mt, pos)
        outproj(range(6, TCH))

    nc.finalize()
    return nc


def _prep_inputs(x, kv_cvec, w_attn, b_attn, w_proj, b_proj):
    x = np.asarray(x, np.float32)
    kv_cvec = np.asarray(kv_cvec, np.float32)
    w_attn = np.asarray(w_attn, np.float32)
    b_attn = np.asarray(b_attn, np.float32)
    w_proj = np.asarray(w_proj, np.float32)
    b_proj = np.asarray(b_proj, np.float32)

    def chunk_major(w):  # [C, N] -> [128, KC, N]
        return np.ascontiguousarray(
            w.reshape(KC, 128, w.shape[1]).transpose(1, 0, 2))

    shared = {
        "wv": chunk_major(w_attn[:, 2 * C:]).astype(np.float16),
        "wp": chunk_major(w_proj).astype(np.float16),
        "b_qk": np.ascontiguousarray(b_attn[:2 * C].reshape(12, 128).T),
        "bv_bc": np.ascontiguousarray(
            np.broadcast_to(b_attn[2 * C:], (128, C))),
        "bp_bc": np.ascontiguousarray(np.broadcast_to(b_proj, (128, C))),
        "tri": (np.arange(128)[:, None] <= np.arange(128)[None, :]
                ).astype(np.float16),
        "ones2": np.kron(np.eye(2), np.ones((1, 64))).astype(np.float16),
    }
    for p in range(NPAIR):
        wqp = np.stack([w_attn[:, 128 * p:128 * p + 128],
                        w_attn[:, C + 128 * p:C + 128 * p + 128]], axis=1)
        shared[f"wq{p}"] = chunk_major(
            wqp.reshape(C, 256)).reshape(128, KC, 2, 128).astype(np.float16)

    in_maps = []
    for b in range(N_CORES):
        kc = kv_cvec[b][:, :C].reshape(PFX, H, D)      # [j, h, d]
        vc = kv_cvec[b][:, C:].reshape(PFX, H, D)
        ktc8 = np.zeros((128, NPAIR, 36), np.float32)
        for s in range(2):
            # [d, p, j] block for head 2p+s
            ktc8[64 * s:64 * s + 64, :, 32 * s:32 * s + 4] = \
                kc[:, s::2, :].transpose(2, 1, 0)
        vcp = np.zeros((64, H, 128), np.float32)
        for s0 in (0, 32):
            vcp[s0:s0 + 4, :, :64] = vc
            vcp[s0:s0 + 4, :, 64] = 1.0
        m = dict(shared)
        m["xt"] = np.ascontiguousarray(
            x[b].T.reshape(KC, 128, T).transpose(1, 0, 2)).astype(np.float16)
        m["kTc8"] = ktc8.astype(np.float16)
        m["vcP"] = vcp.astype(np.float16)
        in_maps.append(m)
    return in_maps


_NC_CACHE = {}


def run_hw(trace=False, **inputs):
    """Build+compile+run on 8 NeuronCores; returns (out [8,1024,768], results)."""
    if "nc" not in _NC_CACHE:
        _NC_CACHE["nc"] = _build()
    nc = _NC_CACHE["nc"]
    in_maps = _prep_inputs(**inputs)
    res = run_bass_kernel_spmd(nc, in_maps, list(range(N_CORES)), trace=trace)
    out = np.stack([res.results[b]["out"].astype(np.float32)
                    for b in range(N_CORES)])
    return out, res


def kernel(**inputs):
    out, _ = run_hw(trace=False, **inputs)
    return out


# revision 75
# speedup vs baseline: 1.0235x; 1.0082x over previous
"""Causal self-attention (12 heads, T=1024, C=768, prefix P=4) on 8 TRN2 cores.

Sharding: data-parallel over batch B=8 -> one batch element per NeuronCore.
No collectives. Weights are replicated to every core.

Per-core kernel (fp16 operands, fp32 psum accumulation):
  qkv projection from chunk-major tiles (xt [128,6,1024], wq [128,6,2,128]):
    qT pair tile [128, T] (head 2p rows 0:64, head 2p+1 rows 64:128),
    kh[s] tiles  [128, T] (head s's k at rows 64s, other half zero),
    v            [128, H, 128] per t-chunk (col 64 = 1.0 -> softmax denom).
  scores^T tile (r, window c): K=128 matmul per head -> psum [128kv, <=512t];
    e2 [128, 2, W] holds both heads' exp(0.125*scores); diagonal band
    tiles get ONE broadcast triangular-mask multiply covering both heads;
    fully-masked columns are never computed nor read.
  prefix scores: ONE matmul per (p, c) via the kTc8 [128, 36] block layout
    (head0 k at rows 0:64 cols 0:4, head1 at rows 64:128 cols 32:36)
    -> pp [36, W] -> single exp; the AV-prefix matmuls (row bases 0/32)
    OPEN each py accumulation group since their exp is ready first.
  AV: py[0:65, t] = [y(64 dims); denom] accumulated over kv chunks.
  norm: DVE copies py -> sb; the two denom rows bounce via DRAM into a
    [64, 16] layout for a lane-parallel reciprocal, return as an f16
    [2, W] row pair, ONE K=2 f16 matmul broadcasts both heads' 1/D
    ([128, W]), DVE multiplies into the f16 yT tiles.
  out = yT.T @ w_proj + b_proj (all f16 operands) -> [T, 768] f16 DMA out.

Schedule: per pair, qkproj(p+1) is emitted between the two AV windows so
the PE has scalar-independent work while the exp queue catches up (exp
throughput on ScalarE is the co-bottleneck). The last pair launches both
denominator chains concurrently and pre-accumulates pairs 0..4 of output
chunks 4-5 in held psum so only tiny pair-5 matmuls trail the final norm.
Startup: warmup matmuls keep the PE HAM unthrottled while the first DMAs
(wq0, per-chunk xt) land; all loads are chunk-major single large-line
transfers.
"""

import numpy as np
from contextlib import ExitStack

import concourse.bass as bass
import concourse.mybir as mybir
import concourse.tile as tile
from concourse import bacc
from concourse.bass_utils import run_bass_kernel_spmd

F32 = mybir.dt.float32
F32R = mybir.dt.float32r
F16 = mybir.dt.float16
N_CORES = 8
T, C, H, D, PFX = 1024, 768, 12, 64, 4
NPAIR = H // 2          # 6 head pairs
KC = C // 128           # 6 contraction chunks
W = 512                 # T window for scores
NW = T // W             # 2 windows
TCH = T // 128          # 8 T chunks
EXP = mybir.ActivationFunctionType.Exp
COPY = mybir.ActivationFunctionType.Copy
SCALE = 1.0 / np.sqrt(D)


def _build():
    nc = bacc.Bacc("TRN2", target_bir_lowering=False, debug=False,
                   num_devices=N_CORES)
    xt_d = nc.declare_dram_parameter("xt", [128, KC, T], F16, isOutput=False)
    wq_d = [nc.declare_dram_parameter(f"wq{p}", [128, KC, 2, 128], F16,
                                      isOutput=False) for p in range(NPAIR)]
    wv_d = nc.declare_dram_parameter("wv", [128, KC, C], F16, isOutput=False)
    wp_d = nc.declare_dram_parameter("wp", [128, KC, C], F16, isOutput=False)
    bqk_d = nc.declare_dram_parameter("b_qk", [128, 12], F32, isOutput=False)
    bv_d = nc.declare_dram_parameter("bv_bc", [128, C], F32, isOutput=False)
    bp_d = nc.declare_dram_parameter("bp_bc", [128, C], F32, isOutput=False)
    ktc_d = nc.declare_dram_parameter("kTc8", [128, NPAIR, 36], F16,
                                      isOutput=False)
    vcp_d = nc.declare_dram_parameter("vcP", [64, H, 128], F16, isOutput=False)
    tri_d = nc.declare_dram_parameter("tri", [128, 128], F16, isOutput=False)
    ones_d = nc.declare_dram_parameter("ones2", [2, 128], F16, isOutput=False)
    out_d = nc.declare_dram_parameter("out", [T, C], F16, isOutput=True)

    with tile.TileContext(nc) as tc, ExitStack() as ctx:
        pers = ctx.enter_context(tc.tile_pool(name="pers", bufs=1))
        wqp = ctx.enter_context(tc.tile_pool(name="wqp", bufs=6))
        qkp = ctx.enter_context(tc.tile_pool(name="qkp", bufs=2))
        khp = ctx.enter_context(tc.tile_pool(name="khp", bufs=4))
        ep = ctx.enter_context(tc.tile_pool(name="ep", bufs=13))
        epp = ctx.enter_context(tc.tile_pool(name="epp", bufs=6))
        sbp = ctx.enter_context(tc.tile_pool(name="sbp", bufs=6))
        rwp = ctx.enter_context(tc.tile_pool(name="rwp", bufs=2))
        drp = ctx.enter_context(tc.tile_pool(name="drp", bufs=2))
        dram = ctx.enter_context(tc.tile_pool(name="dram", bufs=3,
                                              space="DRAM"))
        op = ctx.enter_context(tc.tile_pool(name="op", bufs=2))
        ps = ctx.enter_context(tc.tile_pool(name="ps", bufs=4, space="PSUM"))
        psp = ctx.enter_context(tc.tile_pool(name="psp", bufs=1, space="PSUM"))
        pyp = ctx.enter_context(tc.tile_pool(name="pyp", bufs=2, space="PSUM"))
        pbp = ctx.enter_context(tc.tile_pool(name="pbp", bufs=1, space="PSUM"))

        # ---- PE warmup: HAM needs ~3.4us of activity to unthrottle ------
        wtile = pers.tile([128, W], F16, tag="wtile")
        nc.vector.memset(wtile[:], 0.0)
        pwarm = ps.tile([128, 512], F32, tag="ps", name="pwarm")
        for i in range(20):
            nc.tensor.matmul(pwarm[:], wtile[:, 0:128], wtile[:],
                             start=True, stop=True)

        # ---- persistent loads, priority order ---------------------------
        wq = [None] * NPAIR
        wq[0] = wqp.tile([128, KC, 2, 128], F16, tag="wq", name="wq0")
        nc.sync.dma_start(wq[0][:], wq_d[0][:])
        xt = pers.tile([128, KC, T], F16, tag="xt")
        for k in range(KC):  # per-chunk so the first qkproj matmuls start
            nc.sync.dma_start(xt[:, k, :], xt_d[:, k, :])
        bqk = pers.tile([128, 12], F32, tag="bqk")
        nc.sync.dma_start(bqk[:], bqk_d[:])
        wv = pers.tile([128, KC, C], F16, tag="wv")
        nc.sync.dma_start(wv[:], wv_d[:])
        bv = pers.tile([128, C], F32, tag="bv")
        nc.sync.dma_start(bv[:], bv_d[:])
        tri = pers.tile([128, 128], F16, tag="tri")
        nc.sync.dma_start(tri[:], tri_d[:])
        ktc = pers.tile([128, NPAIR, 36], F16, tag="ktc")
        nc.sync.dma_start(ktc[:], ktc_d[:])
        vcp = pers.tile([64, H, 128], F16, tag="vcp")
        nc.sync.dma_start(vcp[:], vcp_d[:])
        for p in range(1, NPAIR):
            wq[p] = wqp.tile([128, KC, 2, 128], F16, tag="wq", name=f"wq{p}")
            nc.sync.dma_start(wq[p][:], wq_d[p][:])

        wp = pers.tile([128, KC, C], F16, tag="wp")
        nc.sync.dma_start(wp[:], wp_d[:])
        bp = pers.tile([128, C], F32, tag="bp")
        nc.sync.dma_start(bp[:], bp_d[:])

        ones1 = pers.tile([66, 128], F16, tag="ones1")
        nc.sync.dma_start(ones1[64:66, :], ones_d[:])

        yT = [pers.tile([128, T], F16, tag=f"yT{p}", name=f"yT{p}")
              for p in range(NPAIR)]

        # ---- phases ----
        qk_tiles = {}
        ets = {}
        etps = {}
        pys = {}

        def qkproj(p, groups=((0, 0), (0, 1), (1, 0), (1, 1))):
            if p in qk_tiles:
                qT, kh = qk_tiles[p]
            else:
                qT = qkp.tile([128, T], F16, tag="qT", name=f"qT{p}")
                kh = [khp.tile([128, T], F16, tag="kh", name=f"kh{p}_{s}")
                      for s in range(2)]
                qk_tiles[p] = (qT, kh)
                for s in range(2):
                    # head s's k-features live at rows 64s..64s+63 (aligned
                    # with its q rows in the pair tile); other 64 rows zero.
                    nc.gpsimd.memset(kh[s][64 - 64 * s:128 - 64 * s, :], 0.0)
            for half, w in groups:
                pq = ps.tile([128, 512], F32, tag="ps",
                             name=f"pq{p}_{half}_{w}")
                for k in range(KC):
                    nc.tensor.matmul(pq[:], wq[p][:, k, half, :],
                                     xt[:, k, W * w:W * w + W],
                                     start=(k == 0), stop=(k == KC - 1))
                if half == 0:
                    nc.vector.tensor_scalar_add(
                        qT[:, W * w:W * w + W], pq[:], bqk[:, p:p + 1])
                else:
                    for s in range(2):
                        nc.vector.tensor_scalar_add(
                            kh[s][64 * s:64 * s + 64, W * w:W * w + W],
                            pq[64 * s:64 * s + 64, :],
                            bqk[64 * s:64 * s + 64, 6 + p:7 + p])

        def vproj():
            vt = []
            for mt in range(TCH):
                v_ = pers.tile([128, H, 128], F16, tag=f"v{mt}")
                nc.gpsimd.memset(v_[:, :, 64:65], 1.0)
                nc.gpsimd.memset(v_[:, :, 65:128], 0.0)
                for n0, nsz in ((0, 512), (512, 256)):
                    pv = ps.tile([128, 512], F32, tag="ps", name=f"pv{mt}_{n0}")
                    for k in range(KC):
                        nc.tensor.matmul(pv[:, :nsz],
                                         xt[:, k, 128 * mt:128 * mt + 128],
                                         wv[:, k, n0:n0 + nsz],
                                         start=(k == 0), stop=(k == KC - 1))
                    h0, hn = n0 // 64, nsz // 64
                    nc.vector.tensor_add(
                        v_[:, h0:h0 + hn, 0:64],
                        pv[:, :nsz].rearrange("a (h d) -> a h d", d=64),
                        bv[:, n0:n0 + nsz].rearrange("a (h d) -> a h d", d=64))
                vt.append(v_)
            return vt

        def scores(p):
            """Both heads of pair p: concurrent K=64 row-group matmuls."""
            qT, kh = qk_tiles[p]
            for c in range(NW):
                # prefix first: both heads in one matmul via the kTc8 block
                # layout; its exp lands early in the scalar queue so the
                # AV-prefix matmuls never stall on it.
                pp = psp.tile([36, 512], F32, tag="psp", name=f"pp{p}_{c}")
                nc.tensor.matmul(pp[:], ktc[:, p, :],
                                 qT[:, W * c:W * (c + 1)], start=True,
                                 stop=True)
                ep_ = epp.tile([36, W], F16, tag="etp", name=f"etp{p}_{c}")
                nc.scalar.activation(ep_[:], pp[:], EXP, scale=float(SCALE))
                etps[(p, c)] = ep_
                for r in range(4 * c + 4):
                    # both heads' e for (c, r) share one [128, 2, W] tile
                    e2 = ep.tile([128, 2, W], F16, tag="et",
                                 name=f"et{p}_{c}_{r}")
                    ets[(p, c, r)] = e2
                    j0 = 128 * r - W * c if r >= 4 * c else 0
                    for s in range(2):
                        pss = ps.tile([128, 512], F32, tag="ps",
                                      name=f"pss{p}_{s}_{c}_{r}")
                        nc.tensor.matmul(
                            pss[:, j0:W],
                            kh[s][:, 128 * r:128 * r + 128],
                            qT[:, W * c + j0:W * (c + 1)],
                            start=True, stop=True)
                        nc.scalar.activation(e2[:, s, j0:W], pss[:, j0:W],
                                             EXP, scale=float(SCALE))
                    if r >= 4 * c:  # one masked multiply covers both heads
                        nc.vector.tensor_mul(
                            e2[:, :, j0:j0 + 128], e2[:, :, j0:j0 + 128],
                            tri[:].unsqueeze(1).broadcast_to((128, 2, 128)))

        def av(p, c, vt):
            """y^T accumulation for both heads: py[0:65, t] = [y; denom]."""
            py = {}
            for s in range(2):
                py[s] = pyp.tile([128, W], F32, tag="py", name=f"py{p}_{s}_{c}")
                pys[(p, s, c)] = py[s]
            # prefix first: its exp is the earliest scalar op of the window,
            # so these never wait; they open the accumulation group.
            for s in range(2):
                nc.tensor.matmul(py[s][:, :],
                                 vcp[32 * s:32 * s + 4, 2 * p + s, :],
                                 etps[(p, c)][32 * s:32 * s + 4, :],
                                 start=True, stop=False)
            last = 4 * c + 3
            for r in range(4 * c + 4):
                tstart = max(0, 128 * r - W * c)
                for s in range(2):
                    nc.tensor.matmul(py[s][:, tstart:W],
                                     vt[r][:, 2 * p + s, :],
                                     ets[(p, c, r)][:, s, tstart:W],
                                     start=False, stop=(r == last))

        def norm_pre(p, c):
            """Launch the denominator reciprocal chain for window c.

            The two denom rows [1, W] are DMA'd to DRAM, reloaded as
            [64, 16] so the reciprocal runs lane-parallel, converted to
            f16 and DMA'd back as a [2, W] row pair at partitions 64:66."""
            sbs = []
            dstage = dram.tile([2, W], F32, tag="dst", name=f"dst{p}_{c}")
            for s in range(2):
                py = pys[(p, s, c)]
                sb = sbp.tile([65, W], F32, tag="sb", name=f"sb{p}_{s}_{c}")
                nc.vector.tensor_copy(sb[:], py[0:65, :])
                nc.sync.dma_start(dstage[s:s + 1, :], sb[64:65, :])
                sbs.append(sb)
            dT = drp.tile([64, 16], F32, tag="dT", name=f"dT{p}_{c}")
            nc.sync.dma_start(
                dT[:], dstage[:].rearrange("r (q f) -> (r q) f", f=16))
            rT = drp.tile([64, 16], F32, tag="rT", name=f"rT{p}_{c}")
            nc.vector.reciprocal(rT[:], dT[:])
            rT16 = drp.tile([64, 16], F16, tag="rT16", name=f"rF{p}_{c}")
            with nc.allow_low_precision(reason="softmax denom recip f16"):
                nc.vector.tensor_copy(rT16[:], rT[:])
            d2 = dram.tile([2, W], F16, tag="d2", name=f"d2{p}_{c}")
            nc.sync.dma_start(
                d2[:].rearrange("r (q f) -> (r q) f", f=16), rT16[:])
            rrow = rwp.tile([66, W], F16, tag="rrow", name=f"rw{p}_{c}")
            nc.sync.dma_start(rrow[64:66, :], d2[:])
            return sbs, rrow

        def norm_mul(p, c, sbs, rrow):
            """Broadcast 1/D of BOTH heads in one K=2 f16 matmul (rows
            0:64 head 0, rows 64:128 head 1) and multiply into yT."""
            pb = pbp.tile([128, W], F32, tag="pb", name=f"pb{p}_{c}")
            nc.tensor.matmul(pb[:], ones1[64:66, :], rrow[64:66, :],
                             start=True, stop=True)
            for s in range(2):
                nc.vector.tensor_mul(yT[p][64 * s:64 * s + 64,
                                           W * c:W * c + W],
                                     sbs[s][0:64, :],
                                     pb[64 * s:64 * s + 64, :])

        def norm(p, c):
            norm_mul(p, c, *norm_pre(p, c))

        def outproj(mts, kps=range(NPAIR)):
            kps = list(kps)
            for mt in mts:
                osb = op.tile([128, C], F16, tag="osb", name=f"osb{mt}")
                for n0, nsz in ((0, 512), (512, 256)):
                    po = ps.tile([128, 512], F32, tag="ps", name=f"po{mt}_{n0}")
                    for kp in kps:
                        nc.tensor.matmul(po[:, :nsz],
                                         yT[kp][:, 128 * mt:128 * mt + 128],
                                         wp[:, kp, n0:n0 + nsz],
                                         start=(kp == kps[0]),
                                         stop=(kp == kps[-1]))
                    nc.vector.tensor_add(osb[:, n0:n0 + nsz], po[:, :nsz],
                                         bp[:, n0:n0 + nsz])
                nc.sync.dma_start(out_d[128 * mt:128 * mt + 128, :], osb[:])

        def outproj_hold(mt, pool, tag):
            """Accumulate pairs 0..4 for chunk mt, hold the psum open."""
            pos = []
            for n0, nsz in ((0, 512), (512, 256)):
                po = pool.tile([128, 512], F32, tag=tag, name=f"ph{mt}_{n0}")
                for kp in range(NPAIR - 1):
                    nc.tensor.matmul(po[:, :nsz],
                                     yT[kp][:, 128 * mt:128 * mt + 128],
                                     wp[:, kp, n0:n0 + nsz],
                                     start=(kp == 0), stop=False)
                pos.append((n0, nsz, po))
            return pos

        def outproj_finish(mt, pos):
            osb = op.tile([128, C], F16, tag="osb", name=f"osb{mt}")
            for n0, nsz, po in pos:
                nc.tensor.matmul(po[:, :nsz],
                                 yT[NPAIR - 1][:, 128 * mt:128 * mt + 128],
                                 wp[:, NPAIR - 1, n0:n0 + nsz],
                                 start=False, stop=True)
                nc.vector.tensor_add(osb[:, n0:n0 + nsz], po[:, :nsz],
                                     bp[:, n0:n0 + nsz])
            nc.sync.dma_start(out_d[128 * mt:128 * mt + 128, :], osb[:])

        # ---- emission schedule ----
        qkproj(0)
        vt = vproj()
        scores(0)
        for p in range(NPAIR - 1):
            av(p, 0, vt)
            norm(p, 0)
            # scalar-independent PE work here lets the exp queue for
            # window c1 catch up before av(p, 1) consumes it
            qkproj(p + 1)
            av(p, 1, vt)
            norm(p, 1)
            scores(p + 1)
        # last pair: both norm chains launch concurrently; held partial
        # accumulations (pairs 0..4) keep the PE busy while they drain,
        # so only the tiny pair-5 matmuls wait on the final norm.
        pl = NPAIR - 1
        av(pl, 0, vt)
        n0state = norm_pre(pl, 0)
        av(pl, 1, vt)
        n1state = norm_pre(pl, 1)
        norm_mul(pl, 0, *n0state)
        outproj(range(0, 4))
        held = [(4, outproj_hold(4, pyp, "py")),
                (5, outproj_hold(5, ps, "ps"))]
        norm_mul(pl, 1, *n1state)
        for mt, pos in held:
            outproj_finish(mt, pos)
        outproj(range(6, TCH))

    nc.finalize()
    return nc


def _prep_inputs(x, kv_cvec, w_attn, b_attn, w_proj, b_proj):
    x = np.asarray(x, np.float32)
    kv_cvec = np.asarray(kv_cvec, np.float32)
    w_attn = np.asarray(w_attn, np.float32)
    b_attn = np.asarray(b_attn, np.float32)
    w_proj = np.asarray(w_proj, np.float32)
    b_proj = np.asarray(b_proj, np.float32)

    def chunk_major(w):  # [C, N] -> [128, KC, N]
        return np.ascontiguousarray(
            w.reshape(KC, 128, w.shape[1]).transpose(1, 0, 2))

    shared = {
        "wv": chunk_major(w_attn[:, 2 * C:]).astype(np.float16),
        "wp": chunk_major(w_proj).astype(np.float16),
        "b_qk": np.ascontiguousarray(b_attn[:2 * C].reshape(12, 128).T),
        "bv_bc": np.ascontiguousarray(
            np.broadcast_to(b_attn[2 * C:], (128, C))),
        "bp_bc": np.ascontiguousarray(np.broadcast_to(b_proj, (128, C))),
        "tri": (np.arange(128)[:, None] <= np.arange(128)[None, :]
                ).astype(np.float16),
        "ones2": np.kron(np.eye(2), np.ones((1, 64))).astype(np.float16),
    }
    for p in range(NPAIR):
        wqp = np.stack([w_attn[:, 128 * p:128 * p + 128],
                        w_attn[:, C + 128 * p:C + 128 * p + 128]], axis=1)
        shared[f"wq{p}"] = chunk_major(
            wqp.reshape(C, 256)).reshape(128, KC, 2, 128).astype(np.float16)

    in_maps = []
    for b in range(N_CORES):
        kc = kv_cvec[b][:, :C].reshape(PFX, H, D)      # [j, h, d]
        vc = kv_cvec[b][:, C:].reshape(PFX, H, D)
        ktc8 = np.zeros((128, NPAIR, 36), np.float32)
        for s in range(2):
            # [d, p, j] block for head 2p+s
            ktc8[64 * s:64 * s + 64, :, 32 * s:32 * s + 4] = \
                kc[:, s::2, :].transpose(2, 1, 0)
        vcp = np.zeros((64, H, 128), np.float32)
        for s0 in (0, 32):
            vcp[s0:s0 + 4, :, :64] = vc
            vcp[s0:s0 + 4, :, 64] = 1.0
        m = dict(shared)
        m["xt"] = np.ascontiguousarray(
            x[b].T.reshape(KC, 128, T).transpose(1, 0, 2)).astype(np.float16)
        m["kTc8"] = ktc8.astype(np.float16)
        m["vcP"] = vcp.astype(np.float16)
        in_maps.append(m)
    return in_maps


_NC_CACHE = {}


def run_hw(trace=False, **inputs):
    """Build+compile+run on 8 NeuronCores; returns (out [8,1024,768], results)."""
    if "nc" not in _NC_CACHE:
        _NC_CACHE["nc"] = _build()
    nc = _NC_CACHE["nc"]
    in_maps = _prep_inputs(**inputs)
    res = run_bass_kernel_spmd(nc, in_maps, list(range(N_CORES)), trace=trace)
    out = np.stack([res.results[b]["out"].astype(np.float32)
                    for b in range(N_CORES)])
    return out, res


def kernel(**inputs):
    out, _ = run_hw(trace=False, **inputs)
    return out
